# revision 21
# baseline (speedup 1.0000x reference)
"""Multi-head self-attention Trainium2 kernel (8 NeuronCores).

Problem: B=4, S=2048, D=1024, H=8 heads (HD=128).
  qkv = x @ qkv_w.T + qkv_b ; q,k,v = split(qkv)
  q = (q @ q_w.T + q_b)  (same k, v) -> [B,H,S,HD]
  scores = q k^T * HD^-0.5, masked softmax (attn_mask==1 -> -inf), o = attn @ v
  out = o @ out_w.T + out_b

Sharding: 8 cores = 4 batches x 2 head-groups (4 heads each).
Core c: batch b = c % 4, head-group g = c // 4.

Host-side algebraic folding: the qkv projection and per-stream q/k/v
projections are composed into single effective weights (W_eff = w @
qkv_w_slice).  The out-projection is row-parallel across head-groups; the
two partial outputs per batch are summed on host with out_b.

Device flow per core (fp32 PSUM accumulation everywhere):
  q/k projections in fp8e4 DoubleRow (K=256 per matmul): x and W_eff are
    pre-scaled by 8 / 512 into e4m3 on host; the PSUM result (4096x) is
    descaled by the ACT identity that moves it to SBUF.  Softmax noise from
    fp8 q/k is ~1% on attention weights and averages out in o.
  v[S, 4*HD] in bf16 (v feeds o directly: fp8 would cost ~3% output error)
  per head, per q-half (1024 q), software-pipelined 2 chunks deep:
    for kc in 16 k-chunks:
      sT = kT_h[:,kc]^T @ qT_h        [128 k, 1024 q]   (PE -> PSUM f32)
      p  = exp(SCALE * sT)            (ACT -> bf16 SBUF)
      pm = p * keepT[kc]              (DVE; keep = attn_mask.T == 0)
      oT += v[kc]^T-as-lhsT @ pm      -> oT[HD, q]      (PE, PSUM accum)
      pair_j = pm[2j] + pm[2j+1]      (DVE/GpSimd pre-reduction)
      dB += ones^T @ pair_j           16 instead of 32 ones-matmuls
    oT_sb = oT * exp(-ln(dB))         softmax normalization (ACT+DVE -> bf16)
  out_partial[s,:] = sum_h oT_h[:,s_chunk]^T @ outwT_h   (+host bias/sum)
  The half-0 out-projection chunks interleave into half-1 attention so the
  output DMA spreads instead of tailing.
"""

import os
import sys
import types

sys.path.insert(0, "/opt/trn_rl_repo")

import numpy as np
import ml_dtypes

BF16 = ml_dtypes.bfloat16
F8E4 = ml_dtypes.float8_e4m3  # TRN fp8e4: max normal 240

B, S, D, H, HD = 4, 2048, 1024, 8, 128
HG = 2           # head groups
HPG = H // HG    # heads per group (4)
GD = HPG * HD    # dims per group (512)
SCALE = float(HD) ** -0.5
NKC = S // 128   # 16 k chunks
NSC = S // 128   # 16 s chunks
ND = D // 128    # 8 d chunks
NDP = ND // 2    # 4 d-pairs for DoubleRow

X_SCALE = 8.0
W_SCALE = 512.0
PROJ_DESCALE = 1.0 / (X_SCALE * W_SCALE)

# tuning flags
# fp8 DoubleRow delivers the 2x pump once the PE is warm (~213ns for a
# K=256 N=512 matmul); the 427ns early readings were the HAM cold clock
# (1.2GHz), which hits bf16 matmuls exactly the same.  See the warmup spin.
USE_DR = os.environ.get("K_USE_DR", "1") == "1"       # fp8 DoubleRow qk-proj
# GpSimd offload measured net-negative: its SBUF port is an exclusive lock
# shared with DVE, and DVE tensor ops under a concurrent GpSimd ADD run ~2x
# slower (1348ns vs 602ns for the [128,1024] pm multiply).
GPS_PAIRS = int(os.environ.get("K_GPS_PAIRS", "0"))   # denom pairs on GpSimd
OUT_BF16 = os.environ.get("K_OUT_BF16", "1") == "1"

_cached = {}


def _install_ntff_hook_shim():
    """The agent image's antenv lacks axon_hooks; shim it so trace works."""
    if "antenv.axon_hooks" in sys.modules:
        return
    try:
        import trn_agent_boot.trn_boot as _tb

        _hook = _tb._ntff_profile_via_ctypes("/opt/axon/libaxon_pjrt.so")
    except Exception:
        _hook = None
    _m = types.ModuleType("antenv.axon_hooks")
    _m.get_axon_ntff_profile_hook = lambda: _hook
    sys.modules["antenv.axon_hooks"] = _m


def _split_waits(nc, mybir, maxw=1):
    """Walrus in this image allows only one sync wait per instruction;
    hoist extra waits onto preceding NoOps on the same engine."""
    n_new = 0
    for fn in nc.m.functions:
        for bb in fn.blocks:
            newlist = []
            for inst in bb.instructions:
                si = inst.sync_info
                if si is not None and si.on_wait is not None and len(si.on_wait) > maxw:
                    waits = list(si.on_wait)
                    extra, keep = waits[:-maxw], waits[-maxw:]
                    while extra:
                        chunk, extra = extra[:maxw], extra[maxw:]
                        nop = mybir.InstNoOp(name=f"I-waitsplit-{nc.next_id()}")
                        nop.engine = inst.engine
                        nop.sync_info = mybir.SyncInfo(on_wait=chunk, on_update=[])
                        newlist.append(nop)
                        n_new += 1
                    si.on_wait = keep
                newlist.append(inst)
            bb.instructions = newlist
    return n_new


def _build_program(use_vbias=False, use_dr=USE_DR, gps_pairs=GPS_PAIRS,
                   out_bf16=OUT_BF16):
    import concourse.bass as bass
    import concourse.mybir as mybir
    import concourse.tile as tile

    f32 = mybir.dt.float32
    bf16 = mybir.dt.bfloat16
    fp8 = mybir.dt.float8e4
    Exp = mybir.ActivationFunctionType.Exp
    Ident = mybir.ActivationFunctionType.Identity
    Ln = mybir.ActivationFunctionType.Ln
    DR = mybir.MatmulPerfMode.DoubleRow

    nc = bass.Bass()

    # DRAM parameters (per-core shards, pre-tiled on host)
    if use_dr:
        x8 = nc.declare_dram_parameter("x8", [NDP, 128, 2, S], fp8, isOutput=False)
        wq8 = nc.declare_dram_parameter("wq8", [NDP, 128, 2, GD], fp8, isOutput=False)
        wk8 = nc.declare_dram_parameter("wk8", [NDP, 128, 2, GD], fp8, isOutput=False)
    else:
        wqT = nc.declare_dram_parameter("wqT", [ND, 128, GD], bf16, isOutput=False)
        wkT = nc.declare_dram_parameter("wkT", [ND, 128, GD], bf16, isOutput=False)
    xT = nc.declare_dram_parameter("xT", [ND, 128, S], bf16, isOutput=False)
    wvT = nc.declare_dram_parameter("wvT", [ND, 128, GD], bf16, isOutput=False)
    bq = nc.declare_dram_parameter("bq", [128, HPG], f32, isOutput=False)
    bk = nc.declare_dram_parameter("bk", [128, HPG], f32, isOutput=False)
    bvrow = nc.declare_dram_parameter("bvrow", [1, GD], bf16, isOutput=False)
    outwT = nc.declare_dram_parameter("outwT", [HPG, 128, D], bf16, isOutput=False)
    keepT = nc.declare_dram_parameter("keepT", [NKC, 128, S], bf16, isOutput=False)
    out_dt = bf16 if out_bf16 else f32
    out = nc.declare_dram_parameter("out", [S, D], out_dt, isOutput=True)

    with tile.TileContext(nc) as tc:
        import contextlib

        with contextlib.ExitStack() as ctx:
            # --- pools ---
            # big2k rotation (4KB slots): x8(4) + xT(8) + keep(0..3), then
            # keep(4..7) reuse the x8 slots after qk-proj and keep(8..15)
            # the xT slots after v-proj.
            p_big = ctx.enter_context(tc.tile_pool(name="big2k", bufs=16))
            p_pers = ctx.enter_context(tc.tile_pool(name="pers", bufs=1))
            p_pm = ctx.enter_context(tc.tile_pool(name="pm", bufs=10))
            p_acc = ctx.enter_context(tc.tile_pool(name="acc", bufs=6))
            p_sm = ctx.enter_context(tc.tile_pool(name="small", bufs=2))
            pp_big = ctx.enter_context(tc.tile_pool(name="ppbig", bufs=2, space="PSUM"))
            pp_o = ctx.enter_context(tc.tile_pool(name="ppo", bufs=4, space="PSUM"))

            # --- constants ---
            ones128 = p_pers.tile([128, 128], bf16, tag="ones128", name="ones128")
            nc.vector.memset(ones128, 1.0)

            # --- PE warmup spin ---
            # The HAM clock gate keeps the PE at 1.2GHz until it has seen
            # ~3.4us of sustained matmul activity; the real first matmul is
            # DMA-gated to ~8us in, so without this the first ~15us of
            # projection matmuls run at half clock (427ns instead of 213).
            # Spin on the memset constant from t~0 so the array is warm
            # (and stays warm) when the first operands land.
            warm_ps = pp_o.tile([128, 128], f32, tag="ppo", name="warm_ps")
            for _ in range(80):
                nc.tensor.matmul(warm_ps, lhsT=ones128, rhs=ones128,
                                 start=True, stop=True)
            warm_sink = p_sm.tile([128, 128], f32, tag="osb", name="warm_sink")
            nc.vector.tensor_copy(warm_sink, warm_ps)

            # --- DMAs, first-needed first ---
            # All DMAs issue from the sync queue, interleaved weight/x so the
            # first projection group's operands land first.  (Splitting the
            # issue across scalar/gpsimd queues was tried and regressed:
            # first ACT slice moved 18.7->25us and the proj phase starved.)
            x8_tiles = []
            w8 = {}
            if use_dr:
                # q-groups consume x8[dp] at ~3.4us intervals; front-load the
                # x tiles so dp=1 doesn't stall, and batch the small wk8
                # tiles after (k-groups run ~14us later)
                for dp in range(NDP):
                    t = p_pers.tile([128, 2, GD], fp8, tag=f"wq8{dp}", name=f"wq8{dp}")
                    w8[("q", dp)] = t
                    t = p_big.tile([128, 2, S], fp8, tag="big2k", name="big2k")
                    x8_tiles.append(t)
                nc.sync.dma_start(out=w8[("q", 0)], in_=wq8[0])
                nc.sync.dma_start(out=x8_tiles[0], in_=x8[0])
                nc.sync.dma_start(out=x8_tiles[1], in_=x8[1])
                nc.sync.dma_start(out=w8[("q", 1)], in_=wq8[1])
                nc.sync.dma_start(out=x8_tiles[2], in_=x8[2])
                nc.sync.dma_start(out=w8[("q", 2)], in_=wq8[2])
                nc.sync.dma_start(out=x8_tiles[3], in_=x8[3])
                nc.sync.dma_start(out=w8[("q", 3)], in_=wq8[3])
                for dp in range(NDP):
                    t = p_pers.tile([128, 2, GD], fp8, tag=f"wk8{dp}", name=f"wk8{dp}")
                    nc.sync.dma_start(out=t, in_=wk8[dp])
                    w8[("k", dp)] = t
            xt_tiles = []
            if not use_dr:
                for d in range(ND):
                    t = p_pers.tile([128, GD], bf16, tag=f"wq{d}", name=f"wq{d}")
                    nc.sync.dma_start(out=t, in_=wqT[d])
                    w8[("q", d)] = t
                    t = p_big.tile([128, S], bf16, tag="big2k", name="big2k")
                    nc.sync.dma_start(out=t, in_=xT[d])
                    xt_tiles.append(t)

            bq_sb = p_pers.tile([128, HPG], f32, tag="bq", name="bq_sb")
            nc.sync.dma_start(out=bq_sb, in_=bq[:, :])
            bk_sb = p_pers.tile([128, HPG], f32, tag="bk", name="bk_sb")
            nc.sync.dma_start(out=bk_sb, in_=bk[:, :])

            if not use_dr:
                for d in range(ND):
                    t = p_pers.tile([128, GD], bf16, tag=f"wk{d}", name=f"wk{d}")
                    nc.sync.dma_start(out=t, in_=wkT[d])
                    w8[("k", d)] = t
            else:
                for d in range(ND):
                    t = p_big.tile([128, S], bf16, tag="big2k", name="big2k")
                    nc.sync.dma_start(out=t, in_=xT[d])
                    xt_tiles.append(t)
            wv_sb = []
            for d in range(ND):
                t = p_pers.tile([128, GD], bf16, tag=f"wv{d}", name=f"wv{d}")
                nc.sync.dma_start(out=t, in_=wvT[d])
                wv_sb.append(t)

            bv_sb = None
            if use_vbias:
                bv_sb = p_pers.tile([1, GD], bf16, tag="bv", name="bv_sb")
                nc.sync.dma_start(out=bv_sb, in_=bvrow[:, :])

            outw_sb = []
            for h in range(HPG):
                t = p_pers.tile([128, D], bf16, tag=f"outw{h}", name=f"outw{h}")
                nc.sync.dma_start(out=t, in_=outwT[h])
                outw_sb.append(t)

            keep_tiles = [None] * NKC
            for kc in range(4):
                t = p_big.tile([128, S], bf16, tag="big2k", name="big2k")
                nc.sync.dma_start(out=t, in_=keepT[kc])
                keep_tiles[kc] = t

            def keep_sl(kc, lo, hi):
                return keep_tiles[kc][:, lo:hi]

            # --- q/k projections ---
            qT_sb = [p_pers.tile([128, S], bf16, tag=f"qT{h}", name=f"qT{h}") for h in range(HPG)]
            kT_sb = [p_pers.tile([128, S], bf16, tag=f"kT{h}", name=f"kT{h}") for h in range(HPG)]

            if use_dr:
                # per (stream, quarter): 4 per-head psum accumulators over 4
                # d-pairs; group (s, qu) only needs the qu-quarter of x8, so
                # the first matmul gates on wq8[0] + one x8 quarter.
                for sname, dst, bias in (("q", qT_sb, bq_sb), ("k", kT_sb, bk_sb)):
                    for qu in range(4):
                        pss = [
                            pp_o.tile([128, 512], f32, tag="ppo", name="ppo")
                            for _ in range(HPG)
                        ]
                        for dp in range(NDP):
                            rhs = x8_tiles[dp][:, :, qu * 512:(qu + 1) * 512]
                            for h in range(HPG):
                                nc.tensor.matmul(
                                    pss[h],
                                    lhsT=w8[(sname, dp)][:, :, h * 128:(h + 1) * 128],
                                    rhs=rhs,
                                    start=(dp == 0),
                                    stop=(dp == NDP - 1),
                                    perf_mode=DR,
                                )
                        for h in range(HPG):
                            nc.scalar.activation(
                                out=dst[h][:, qu * 512:(qu + 1) * 512],
                                in_=pss[h],
                                func=Ident,
                                bias=bias[:, h:h + 1],
                                scale=PROJ_DESCALE,
                            )
            else:
                for h in range(HPG):
                    for sname, dst, bias in (("q", qT_sb, bq_sb), ("k", kT_sb, bk_sb)):
                        pss = [
                            pp_o.tile([128, 512], f32, tag="ppo", name="ppo")
                            for _ in range(4)
                        ]
                        for d in range(ND):
                            lhs = w8[(sname, d)][:, h * 128:(h + 1) * 128]
                            for qu in range(4):
                                nc.tensor.matmul(
                                    pss[qu],
                                    lhsT=lhs,
                                    rhs=xt_tiles[d][:, qu * 512:(qu + 1) * 512],
                                    start=(d == 0),
                                    stop=(d == ND - 1),
                                )
                        for qu in range(4):
                            nc.scalar.activation(
                                out=dst[h][:, qu * 512:(qu + 1) * 512],
                                in_=pss[qu],
                                func=Ident,
                                bias=bias[:, h:h + 1],
                            )

            # keep(4..7) into the freed x8 slots (or into the rotation after
            # the bf16 path's first four reuses)
            for kc in range(4, 8):
                t = p_big.tile([128, S], bf16, tag="big2k", name="big2k")
                nc.sync.dma_start(out=t, in_=keepT[kc])
                keep_tiles[kc] = t

            # --- v projection (bf16; fp8 v would cost ~3% output error) ---
            v_sb = [p_pers.tile([128, GD], bf16, tag=f"v{sc}", name=f"v{sc}") for sc in range(NSC)]
            for sc in range(NSC):
                ps = pp_o.tile([128, GD], f32, tag="ppo", name="ppo")
                for d in range(ND):
                    nc.tensor.matmul(
                        ps,
                        lhsT=xt_tiles[d][:, sc * 128:(sc + 1) * 128],
                        rhs=wv_sb[d],
                        start=(d == 0),
                        stop=(d == ND - 1) and not use_vbias,
                    )
                if use_vbias:
                    nc.tensor.matmul(
                        ps,
                        lhsT=ones128[0:1, :],
                        rhs=bv_sb,
                        start=False,
                        stop=True,
                    )
                nc.vector.tensor_copy(v_sb[sc], ps)

            # keep(8..15) into the freed xT slots
            for kc in range(8, NKC):
                t = p_big.tile([128, S], bf16, tag="big2k", name="big2k")
                nc.sync.dma_start(out=t, in_=keepT[kc])
                keep_tiles[kc] = t

            # --- attention (half-major so half-0 out-projection can
            # interleave into half-1) + out-projection ---
            oT_sb = [p_pers.tile([128, S], bf16, tag=f"oT{h}", name=f"oT{h}") for h in range(HPG)]

            osb_dt = bf16 if out_bf16 else f32

            def out_proj(sc, on_act=False):
                ps = pp_big.tile([128, 1024], f32, tag="ppbig", name="ppbig")
                for h in range(HPG):
                    for nn in range(2):
                        nc.tensor.matmul(
                            ps[:, nn * 512:(nn + 1) * 512],
                            lhsT=oT_sb[h][:, sc * 128:(sc + 1) * 128],
                            rhs=outw_sb[h][:, nn * 512:(nn + 1) * 512],
                            start=(h == 0),
                            stop=(h == HPG - 1),
                        )
                osb = p_sm.tile([128, 1024], osb_dt, tag="osb", name="osb")
                if on_act:
                    # final 8 chunks: ACT is idle in the tail and the DVE
                    # casts otherwise serialize behind the PSUM rotation
                    nc.scalar.copy(out=osb, in_=ps)
                else:
                    nc.vector.tensor_copy(osb, ps)
                nc.sync.dma_start(out=out[sc * 128:(sc + 1) * 128, :], in_=osb)

            def attention_hh(h, half, interleave, finish_prev=None):
                """interleave: sc chunks to out-project after this hh.
                finish_prev: the previous hh's deferred softmax-normalize,
                emitted after this hh's kc=2 so the boundary exps run
                back-to-back on ACT (emitting ln/exp between halves starved
                the sT rotation and stalled the PE ~1.3us, 4x per hh)."""
                q0 = half * 1024
                o_ps = [pp_o.tile([128, 512], f32, tag="ppo", name="ppo") for _ in range(2)]
                d_ps = [pp_o.tile([128, 512], f32, tag="ppo", name="ppo") for _ in range(2)]

                def consume(kc, pm):
                    for qq in range(2):
                        nc.tensor.matmul(
                            o_ps[qq],
                            lhsT=v_sb[kc][:, h * 128:(h + 1) * 128],
                            rhs=pm[:, qq * 512:(qq + 1) * 512],
                            start=(kc == 0),
                            stop=(kc == NKC - 1),
                        )

                def d_mm(pr, pacc):
                    for qq in range(2):
                        nc.tensor.matmul(
                            d_ps[qq],
                            lhsT=ones128,
                            rhs=pacc[:, qq * 512:(qq + 1) * 512],
                            start=(pr == 0),
                            stop=(pr == NKC // 2 - 1),
                        )

                pending = []      # [(kc, pm)] — 2-stage consume delay
                pairs = [None] * (NKC // 2)   # accumulated pm pairs
                d_emit = []       # pairs ready to ones-matmul
                pm_even = None
                for kc in range(NKC):
                    sT = pp_big.tile([128, 1024], f32, tag="ppbig", name="ppbig")
                    for nn in range(2):
                        nc.tensor.matmul(
                            sT[:, nn * 512:(nn + 1) * 512],
                            lhsT=kT_sb[h][:, kc * 128:(kc + 1) * 128],
                            rhs=qT_sb[h][:, q0 + nn * 512:q0 + (nn + 1) * 512],
                            start=True,
                            stop=True,
                        )
                    p = p_pm.tile([128, 1024], bf16, tag="pm", name="pm")
                    nc.scalar.activation(out=p, in_=sT, func=Exp, scale=SCALE)
                    pm = p_pm.tile([128, 1024], bf16, tag="pm", name="pm")
                    nc.vector.tensor_mul(pm, p, keep_sl(kc, q0, q0 + 1024))

                    if kc % 2 == 0:
                        pm_even = pm
                    else:
                        pr = kc // 2
                        pacc = p_acc.tile([128, 1024], bf16, tag="acc", name="acc")
                        eng = nc.gpsimd if pr < gps_pairs else nc.vector
                        eng.tensor_add(pacc, pm_even, pm)
                        pairs[pr] = pacc
                        d_emit.append(pr)

                    pending.append((kc, pm))
                    if len(pending) > 4:
                        consume(*pending.pop(0))
                    # ones-matmul a pair two k-chunks after it was formed so
                    # the PE never waits on the DVE adds
                    if d_emit and d_emit[0] <= (kc - 3) // 2:
                        pr = d_emit.pop(0)
                        d_mm(pr, pairs[pr])
                    if kc == 2 and finish_prev is not None:
                        finish_prev()
                    # previous half's out-projection chunks, mid-loop: fills
                    # ACT-lag windows with PE work, and keeps their DVE cast
                    # out of the boundary sT rotation (was a 1.25us PE stall)
                    if interleave and kc in (6, 11):
                        out_proj(interleave.pop(0))
                for item in pending:
                    consume(*item)
                for pr in d_emit:
                    d_mm(pr, pairs[pr])
                while interleave:
                    out_proj(interleave.pop(0))

                def finish():
                    for qq in range(2):
                        # 1/d via exp(-ln(d)) on ACT: frees the PSUM
                        # accumulators fast and keeps DVE reciprocal (which
                        # measures ~6 cyc/elem) off the critical path.
                        lnd = p_sm.tile([128, 512], f32, tag="lnd", name="lnd")
                        nc.scalar.activation(out=lnd, in_=d_ps[qq], func=Ln)
                        rdb = p_sm.tile([128, 512], f32, tag="rdb", name="rdb")
                        nc.scalar.activation(out=rdb, in_=lnd, func=Exp, scale=-1.0)
                        nc.vector.tensor_mul(
                            oT_sb[h][:, q0 + qq * 512:q0 + (qq + 1) * 512],
                            o_ps[qq],
                            rdb,
                        )

                return finish

            fin = None
            for h in range(HPG):
                fin = attention_hh(h, 0, [], fin)
            sc_queue = list(range(8))
            for h in range(HPG):
                fin = attention_hh(h, 1, [sc_queue.pop(0), sc_queue.pop(0)], fin)
            fin()
            for sc in range(8, NSC):
                out_proj(sc, on_act=(sc % 2 == 1))

    _split_waits(nc, mybir, maxw=1)
    return nc


def _prep_core_inputs(x, attn_mask, qkv_w, qkv_b, q_w, q_b, k_w, k_b, v_w, v_b,
                      out_w, use_dr=USE_DR):
    """Host-side: fold projections, shard, pre-transpose/tile, cast."""
    f = np.float32
    x = np.asarray(x, f)
    qkv_w = np.asarray(qkv_w, f)
    qkv_b = np.asarray(qkv_b, f)
    Ws = {}
    bs = {}
    for i, (w, b) in enumerate(((q_w, q_b), (k_w, k_b), (v_w, v_b))):
        w = np.asarray(w, f)
        b = np.asarray(b, f)
        sl = slice(i * D, (i + 1) * D)
        Ws[i] = w @ qkv_w[sl]              # [D, D] effective
        bs[i] = b + w @ qkv_b[sl]          # [D]
    out_wT = np.ascontiguousarray(np.asarray(out_w, f).T)  # [D(hd), D(model)]

    keepT = (np.asarray(attn_mask).T == 0).astype(BF16)    # [k, q]
    keepT_t = np.ascontiguousarray(keepT).reshape(NKC, 128, S)

    def dr_pack(mat_T, scale):
        # mat_T: [D, N] (contraction-major) -> [NDP, 128, 2, N] e4m3 * scale
        m = (mat_T * scale).reshape(NDP, 2, 128, -1).transpose(0, 2, 1, 3)
        return np.ascontiguousarray(m.astype(F8E4))

    xT_all = []
    x8_all = []
    for b_i in range(B):
        xb = np.ascontiguousarray(x[b_i].T)                # [D, S] f32
        xT_all.append(np.ascontiguousarray(xb.astype(BF16)).reshape(ND, 128, S))
        if use_dr:
            x8_all.append(dr_pack(xb, X_SCALE))

    maps = []
    for c in range(8):
        b_i = c % B
        g = c // B
        sl = slice(g * GD, (g + 1) * GD)
        m = {
            "xT": xT_all[b_i],
            "wvT": np.ascontiguousarray(Ws[2][sl].T.astype(BF16)).reshape(ND, 128, GD),
            "bq": np.ascontiguousarray(bs[0][sl].reshape(HPG, 128).T.astype(f)),
            "bk": np.ascontiguousarray(bs[1][sl].reshape(HPG, 128).T.astype(f)),
            "bvrow": bs[2][sl].astype(BF16).reshape(1, GD),
            "outwT": np.ascontiguousarray(out_wT[sl].astype(BF16)).reshape(HPG, 128, D),
            "keepT": keepT_t,
        }
        if use_dr:
            m["x8"] = x8_all[b_i]
            m["wq8"] = dr_pack(np.ascontiguousarray(Ws[0][sl].T), W_SCALE)
            m["wk8"] = dr_pack(np.ascontiguousarray(Ws[1][sl].T), W_SCALE)
        else:
            m["wqT"] = np.ascontiguousarray(Ws[0][sl].T.astype(BF16)).reshape(ND, 128, GD)
            m["wkT"] = np.ascontiguousarray(Ws[1][sl].T.astype(BF16)).reshape(ND, 128, GD)
        maps.append(m)
    return maps


def kernel(x, attn_mask, qkv_w, qkv_b, q_w, q_b, k_w, k_b, v_w, v_b,
           out_w, out_b, _trace=False):
    _install_ntff_hook_shim()
    from concourse.bass_utils import run_bass_kernel_spmd

    in_maps = _prep_core_inputs(
        x, attn_mask, qkv_w, qkv_b, q_w, q_b, k_w, k_b, v_w, v_b, out_w
    )
    use_vbias = bool(np.any(np.asarray(in_maps[0]["bvrow"], np.float32) != 0))
    key = ("nc", use_vbias)
    if key not in _cached:
        _cached[key] = _build_program(use_vbias=use_vbias)
    nc = _cached[key]
    core_ids = list(range(8))
    try:
        res = run_bass_kernel_spmd(nc, in_maps, core_ids, trace=_trace)
    except Exception:
        # transient NRT device wedge recovers on retry
        res = run_bass_kernel_spmd(nc, in_maps, core_ids, trace=_trace)
    _cached["last_result"] = res

    out_b = np.asarray(out_b, np.float32)
    full = np.empty((B, S, D), np.float32)
    for b_i in range(B):
        full[b_i] = (
            res.results[b_i]["out"].astype(np.float32)
            + res.results[b_i + B]["out"].astype(np.float32)
            + out_b
        )
    return full


# revision 24
# speedup vs baseline: 1.0078x; 1.0078x over previous
"""Multi-head self-attention Trainium2 kernel (8 NeuronCores).

Problem: B=4, S=2048, D=1024, H=8 heads (HD=128).
  qkv = x @ qkv_w.T + qkv_b ; q,k,v = split(qkv)
  q = (q @ q_w.T + q_b)  (same k, v) -> [B,H,S,HD]
  scores = q k^T * HD^-0.5, masked softmax (attn_mask==1 -> -inf), o = attn @ v
  out = o @ out_w.T + out_b

Sharding: 8 cores = 4 batches x 2 head-groups (4 heads each).
Core c: batch b = c % 4, head-group g = c // 4.

Host-side algebraic folding: the qkv projection and per-stream q/k/v
projections are composed into single effective weights (W_eff = w @
qkv_w_slice).  The out-projection is row-parallel across head-groups; the
two partial outputs per batch are summed on host with out_b.

Device flow per core (fp32 PSUM accumulation everywhere):
  q/k projections in fp8e4 DoubleRow (K=256 per matmul): x and W_eff are
    pre-scaled by 8 / 512 into e4m3 on host; the PSUM result (4096x) is
    descaled by the ACT identity that moves it to SBUF.  Softmax noise from
    fp8 q/k is ~1% on attention weights and averages out in o.
  v[S, 4*HD] in bf16 (v feeds o directly: fp8 would cost ~3% output error)
  per head, per q-half (1024 q), software-pipelined 2 chunks deep:
    for kc in 16 k-chunks:
      sT = kT_h[:,kc]^T @ qT_h        [128 k, 1024 q]   (PE -> PSUM f32)
      p  = exp(SCALE * sT)            (ACT -> bf16 SBUF)
      pm = p * keepT[kc]              (DVE; keep = attn_mask.T == 0)
      oT += v[kc]^T-as-lhsT @ pm      -> oT[HD, q]      (PE, PSUM accum)
      pair_j = pm[2j] + pm[2j+1]      (DVE/GpSimd pre-reduction)
      dB += ones^T @ pair_j           16 instead of 32 ones-matmuls
    oT_sb = oT * exp(-ln(dB))         softmax normalization (ACT+DVE -> bf16)
  out_partial[s,:] = sum_h oT_h[:,s_chunk]^T @ outwT_h   (+host bias/sum)
  The half-0 out-projection chunks interleave into half-1 attention so the
  output DMA spreads instead of tailing.
"""

import os
import sys
import types

sys.path.insert(0, "/opt/trn_rl_repo")

import numpy as np
import ml_dtypes

BF16 = ml_dtypes.bfloat16
F8E4 = ml_dtypes.float8_e4m3  # TRN fp8e4: max normal 240

B, S, D, H, HD = 4, 2048, 1024, 8, 128
HG = 2           # head groups
HPG = H // HG    # heads per group (4)
GD = HPG * HD    # dims per group (512)
SCALE = float(HD) ** -0.5
NKC = S // 128   # 16 k chunks
NSC = S // 128   # 16 s chunks
ND = D // 128    # 8 d chunks
NDP = ND // 2    # 4 d-pairs for DoubleRow

X_SCALE = 8.0
W_SCALE = 512.0
PROJ_DESCALE = 1.0 / (X_SCALE * W_SCALE)

# tuning flags
# fp8 DoubleRow delivers the 2x pump once the PE is warm (~213ns for a
# K=256 N=512 matmul); the 427ns early readings were the HAM cold clock
# (1.2GHz), which hits bf16 matmuls exactly the same.  See the warmup spin.
USE_DR = os.environ.get("K_USE_DR", "1") == "1"       # fp8 DoubleRow qk-proj
# GpSimd offload measured net-negative: its SBUF port is an exclusive lock
# shared with DVE, and DVE tensor ops under a concurrent GpSimd ADD run ~2x
# slower (1348ns vs 602ns for the [128,1024] pm multiply).
GPS_PAIRS = int(os.environ.get("K_GPS_PAIRS", "0"))   # denom pairs on GpSimd
OUT_BF16 = os.environ.get("K_OUT_BF16", "1") == "1"

_cached = {}


def _install_ntff_hook_shim():
    """The agent image's antenv lacks axon_hooks; shim it so trace works."""
    if "antenv.axon_hooks" in sys.modules:
        return
    try:
        import trn_agent_boot.trn_boot as _tb

        _hook = _tb._ntff_profile_via_ctypes("/opt/axon/libaxon_pjrt.so")
    except Exception:
        _hook = None
    _m = types.ModuleType("antenv.axon_hooks")
    _m.get_axon_ntff_profile_hook = lambda: _hook
    sys.modules["antenv.axon_hooks"] = _m


def _split_waits(nc, mybir, maxw=1):
    """Walrus in this image allows only one sync wait per instruction;
    hoist extra waits onto preceding NoOps on the same engine."""
    n_new = 0
    for fn in nc.m.functions:
        for bb in fn.blocks:
            newlist = []
            for inst in bb.instructions:
                si = inst.sync_info
                if si is not None and si.on_wait is not None and len(si.on_wait) > maxw:
                    waits = list(si.on_wait)
                    extra, keep = waits[:-maxw], waits[-maxw:]
                    while extra:
                        chunk, extra = extra[:maxw], extra[maxw:]
                        nop = mybir.InstNoOp(name=f"I-waitsplit-{nc.next_id()}")
                        nop.engine = inst.engine
                        nop.sync_info = mybir.SyncInfo(on_wait=chunk, on_update=[])
                        newlist.append(nop)
                        n_new += 1
                    si.on_wait = keep
                newlist.append(inst)
            bb.instructions = newlist
    return n_new


def _build_program(use_vbias=False, use_dr=USE_DR, gps_pairs=GPS_PAIRS,
                   out_bf16=OUT_BF16):
    import concourse.bass as bass
    import concourse.mybir as mybir
    import concourse.tile as tile

    f32 = mybir.dt.float32
    bf16 = mybir.dt.bfloat16
    fp8 = mybir.dt.float8e4
    Exp = mybir.ActivationFunctionType.Exp
    Ident = mybir.ActivationFunctionType.Identity
    Ln = mybir.ActivationFunctionType.Ln
    DR = mybir.MatmulPerfMode.DoubleRow

    nc = bass.Bass()

    # DRAM parameters (per-core shards, pre-tiled on host)
    if use_dr:
        x8 = nc.declare_dram_parameter("x8", [NDP, 128, 2, S], fp8, isOutput=False)
        wq8 = nc.declare_dram_parameter("wq8", [NDP, 128, 2, GD], fp8, isOutput=False)
        wk8 = nc.declare_dram_parameter("wk8", [NDP, 128, 2, GD], fp8, isOutput=False)
    else:
        wqT = nc.declare_dram_parameter("wqT", [ND, 128, GD], bf16, isOutput=False)
        wkT = nc.declare_dram_parameter("wkT", [ND, 128, GD], bf16, isOutput=False)
    xT = nc.declare_dram_parameter("xT", [ND, 128, S], bf16, isOutput=False)
    wvT = nc.declare_dram_parameter("wvT", [ND, 128, GD], bf16, isOutput=False)
    bq = nc.declare_dram_parameter("bq", [128, HPG], f32, isOutput=False)
    bk = nc.declare_dram_parameter("bk", [128, HPG], f32, isOutput=False)
    bvrow = nc.declare_dram_parameter("bvrow", [1, GD], bf16, isOutput=False)
    outwT = nc.declare_dram_parameter("outwT", [HPG, 128, D], bf16, isOutput=False)
    keepT = nc.declare_dram_parameter("keepT", [NKC, 128, S], bf16, isOutput=False)
    out_dt = bf16 if out_bf16 else f32
    out = nc.declare_dram_parameter("out", [S, D], out_dt, isOutput=True)

    with tile.TileContext(nc) as tc:
        import contextlib

        with contextlib.ExitStack() as ctx:
            # --- pools ---
            # big2k rotation (4KB slots): x8(4) + xT(8) + keep(0..3), then
            # keep(4..7) reuse the x8 slots after qk-proj and keep(8..15)
            # the xT slots after v-proj.
            p_big = ctx.enter_context(tc.tile_pool(name="big2k", bufs=16))
            p_pers = ctx.enter_context(tc.tile_pool(name="pers", bufs=1))
            p_pm = ctx.enter_context(tc.tile_pool(name="pm", bufs=10))
            p_acc = ctx.enter_context(tc.tile_pool(name="acc", bufs=6))
            p_sm = ctx.enter_context(tc.tile_pool(name="small", bufs=2))
            pp_big = ctx.enter_context(tc.tile_pool(name="ppbig", bufs=2, space="PSUM"))
            pp_o = ctx.enter_context(tc.tile_pool(name="ppo", bufs=4, space="PSUM"))

            # --- constants ---
            ones128 = p_pers.tile([128, 128], bf16, tag="ones128", name="ones128")
            nc.vector.memset(ones128, 1.0)

            # --- PE warmup spin ---
            # The HAM clock gate keeps the PE at 1.2GHz until it has seen
            # ~3.4us of sustained matmul activity; the real first matmul is
            # DMA-gated to ~8us in, so without this the first ~15us of
            # projection matmuls run at half clock (427ns instead of 213).
            # Spin on the memset constant from t~0 so the array is warm
            # (and stays warm) when the first operands land.
            warm_ps = pp_o.tile([128, 128], f32, tag="ppo", name="warm_ps")
            for _ in range(80):
                nc.tensor.matmul(warm_ps, lhsT=ones128, rhs=ones128,
                                 start=True, stop=True)
            warm_sink = p_sm.tile([128, 128], f32, tag="osb", name="warm_sink")
            nc.vector.tensor_copy(warm_sink, warm_ps)

            # --- DMAs, first-needed first ---
            # All DMAs issue from the sync queue, interleaved weight/x so the
            # first projection group's operands land first.  (Splitting the
            # issue across scalar/gpsimd queues was tried and regressed:
            # first ACT slice moved 18.7->25us and the proj phase starved.)
            x8_tiles = []
            w8 = {}
            if use_dr:
                # q-groups consume x8[dp] at ~3.4us intervals; front-load the
                # x tiles so dp=1 doesn't stall, and batch the small wk8
                # tiles after (k-groups run ~14us later)
                for dp in range(NDP):
                    t = p_pers.tile([128, 2, GD], fp8, tag=f"wq8{dp}", name=f"wq8{dp}")
                    w8[("q", dp)] = t
                    t = p_big.tile([128, 2, S], fp8, tag="big2k", name="big2k")
                    x8_tiles.append(t)
                nc.sync.dma_start(out=w8[("q", 0)], in_=wq8[0])
                nc.sync.dma_start(out=x8_tiles[0], in_=x8[0])
                nc.sync.dma_start(out=x8_tiles[1], in_=x8[1])
                nc.sync.dma_start(out=w8[("q", 1)], in_=wq8[1])
                nc.sync.dma_start(out=x8_tiles[2], in_=x8[2])
                nc.sync.dma_start(out=w8[("q", 2)], in_=wq8[2])
                nc.sync.dma_start(out=x8_tiles[3], in_=x8[3])
                nc.sync.dma_start(out=w8[("q", 3)], in_=wq8[3])
                for dp in range(NDP):
                    t = p_pers.tile([128, 2, GD], fp8, tag=f"wk8{dp}", name=f"wk8{dp}")
                    nc.sync.dma_start(out=t, in_=wk8[dp])
                    w8[("k", dp)] = t
            xt_tiles = []
            if not use_dr:
                for d in range(ND):
                    t = p_pers.tile([128, GD], bf16, tag=f"wq{d}", name=f"wq{d}")
                    nc.sync.dma_start(out=t, in_=wqT[d])
                    w8[("q", d)] = t
                    t = p_big.tile([128, S], bf16, tag="big2k", name="big2k")
                    nc.sync.dma_start(out=t, in_=xT[d])
                    xt_tiles.append(t)

            bq_sb = p_pers.tile([128, HPG], f32, tag="bq", name="bq_sb")
            nc.sync.dma_start(out=bq_sb, in_=bq[:, :])
            bk_sb = p_pers.tile([128, HPG], f32, tag="bk", name="bk_sb")
            nc.sync.dma_start(out=bk_sb, in_=bk[:, :])

            if not use_dr:
                for d in range(ND):
                    t = p_pers.tile([128, GD], bf16, tag=f"wk{d}", name=f"wk{d}")
                    nc.sync.dma_start(out=t, in_=wkT[d])
                    w8[("k", d)] = t
            else:
                for d in range(ND):
                    t = p_big.tile([128, S], bf16, tag="big2k", name="big2k")
                    nc.sync.dma_start(out=t, in_=xT[d])
                    xt_tiles.append(t)
            wv_sb = []
            for d in range(ND):
                t = p_pers.tile([128, GD], bf16, tag=f"wv{d}", name=f"wv{d}")
                nc.sync.dma_start(out=t, in_=wvT[d])
                wv_sb.append(t)

            bv_sb = None
            if use_vbias:
                bv_sb = p_pers.tile([1, GD], bf16, tag="bv", name="bv_sb")
                nc.sync.dma_start(out=bv_sb, in_=bvrow[:, :])

            outw_sb = []
            for h in range(HPG):
                t = p_pers.tile([128, D], bf16, tag=f"outw{h}", name=f"outw{h}")
                nc.sync.dma_start(out=t, in_=outwT[h])
                outw_sb.append(t)

            keep_tiles = [None] * NKC
            for kc in range(4):
                t = p_big.tile([128, S], bf16, tag="big2k", name="big2k")
                nc.sync.dma_start(out=t, in_=keepT[kc])
                keep_tiles[kc] = t

            def keep_sl(kc, lo, hi):
                return keep_tiles[kc][:, lo:hi]

            # --- q/k projections ---
            qT_sb = [p_pers.tile([128, S], bf16, tag=f"qT{h}", name=f"qT{h}") for h in range(HPG)]
            kT_sb = [p_pers.tile([128, S], bf16, tag=f"kT{h}", name=f"kT{h}") for h in range(HPG)]

            if use_dr:
                # per (stream, quarter): 4 per-head psum accumulators over 4
                # d-pairs; group (s, qu) only needs the qu-quarter of x8, so
                # the first matmul gates on wq8[0] + one x8 quarter.
                for sname, dst, bias in (("q", qT_sb, bq_sb), ("k", kT_sb, bk_sb)):
                    for qu in range(4):
                        pss = [
                            pp_o.tile([128, 512], f32, tag="ppo", name="ppo")
                            for _ in range(HPG)
                        ]
                        for dp in range(NDP):
                            rhs = x8_tiles[dp][:, :, qu * 512:(qu + 1) * 512]
                            for h in range(HPG):
                                nc.tensor.matmul(
                                    pss[h],
                                    lhsT=w8[(sname, dp)][:, :, h * 128:(h + 1) * 128],
                                    rhs=rhs,
                                    start=(dp == 0),
                                    stop=(dp == NDP - 1),
                                    perf_mode=DR,
                                )
                        for h in range(HPG):
                            nc.scalar.activation(
                                out=dst[h][:, qu * 512:(qu + 1) * 512],
                                in_=pss[h],
                                func=Ident,
                                bias=bias[:, h:h + 1],
                                scale=PROJ_DESCALE,
                            )
            else:
                for h in range(HPG):
                    for sname, dst, bias in (("q", qT_sb, bq_sb), ("k", kT_sb, bk_sb)):
                        pss = [
                            pp_o.tile([128, 512], f32, tag="ppo", name="ppo")
                            for _ in range(4)
                        ]
                        for d in range(ND):
                            lhs = w8[(sname, d)][:, h * 128:(h + 1) * 128]
                            for qu in range(4):
                                nc.tensor.matmul(
                                    pss[qu],
                                    lhsT=lhs,
                                    rhs=xt_tiles[d][:, qu * 512:(qu + 1) * 512],
                                    start=(d == 0),
                                    stop=(d == ND - 1),
                                )
                        for qu in range(4):
                            nc.scalar.activation(
                                out=dst[h][:, qu * 512:(qu + 1) * 512],
                                in_=pss[qu],
                                func=Ident,
                                bias=bias[:, h:h + 1],
                            )

            # keep(4..7) into the freed x8 slots (or into the rotation after
            # the bf16 path's first four reuses)
            for kc in range(4, 8):
                t = p_big.tile([128, S], bf16, tag="big2k", name="big2k")
                nc.sync.dma_start(out=t, in_=keepT[kc])
                keep_tiles[kc] = t

            # --- v projection (bf16; fp8 v would cost ~3% output error) ---
            v_sb = [p_pers.tile([128, GD], bf16, tag=f"v{sc}", name=f"v{sc}") for sc in range(NSC)]
            for sc in range(NSC):
                ps = pp_o.tile([128, GD], f32, tag="ppo", name="ppo")
                for d in range(ND):
                    nc.tensor.matmul(
                        ps,
                        lhsT=xt_tiles[d][:, sc * 128:(sc + 1) * 128],
                        rhs=wv_sb[d],
                        start=(d == 0),
                        stop=(d == ND - 1) and not use_vbias,
                    )
                if use_vbias:
                    nc.tensor.matmul(
                        ps,
                        lhsT=ones128[0:1, :],
                        rhs=bv_sb,
                        start=False,
                        stop=True,
                    )
                nc.vector.tensor_copy(v_sb[sc], ps)

            # keep(8..15) into the freed xT slots
            for kc in range(8, NKC):
                t = p_big.tile([128, S], bf16, tag="big2k", name="big2k")
                nc.sync.dma_start(out=t, in_=keepT[kc])
                keep_tiles[kc] = t

            # --- attention (half-major so half-0 out-projection can
            # interleave into half-1) + out-projection ---
            oT_sb = [p_pers.tile([128, S], bf16, tag=f"oT{h}", name=f"oT{h}") for h in range(HPG)]

            osb_dt = bf16 if out_bf16 else f32

            def out_proj(sc, on_act=False, tail=False):
                # tail blocks run after attention, when the 4-buf pp_o pool
                # is idle: using it there (2x[128,512] per chunk) gives the
                # rotation 2x the slack of pp_big and drops the ~1.5us
                # stalls between the final chunks.
                if tail:
                    pss = [pp_o.tile([128, 512], f32, tag="ppo", name="ppo")
                           for _ in range(2)]
                else:
                    ps = pp_big.tile([128, 1024], f32, tag="ppbig", name="ppbig")
                    pss = [ps[:, 0:512], ps[:, 512:1024]]
                for h in range(HPG):
                    for nn in range(2):
                        nc.tensor.matmul(
                            pss[nn],
                            lhsT=oT_sb[h][:, sc * 128:(sc + 1) * 128],
                            rhs=outw_sb[h][:, nn * 512:(nn + 1) * 512],
                            start=(h == 0),
                            stop=(h == HPG - 1),
                        )
                osb = p_sm.tile([128, 1024], osb_dt, tag="osb", name="osb")
                for nn in range(2):
                    half = osb[:, nn * 512:(nn + 1) * 512]
                    if on_act and nn == 1:
                        # split the tail cast across ACT (idle then) and DVE
                        nc.scalar.copy(out=half, in_=pss[nn])
                    else:
                        nc.vector.tensor_copy(half, pss[nn])
                nc.sync.dma_start(out=out[sc * 128:(sc + 1) * 128, :], in_=osb)

            def attention_hh(h, half, interleave, finish_prev=None):
                """interleave: sc chunks to out-project after this hh.
                finish_prev: the previous hh's deferred softmax-normalize,
                emitted after this hh's kc=2 so the boundary exps run
                back-to-back on ACT (emitting ln/exp between halves starved
                the sT rotation and stalled the PE ~1.3us, 4x per hh)."""
                q0 = half * 1024
                o_ps = [pp_o.tile([128, 512], f32, tag="ppo", name="ppo") for _ in range(2)]
                d_ps = [pp_o.tile([128, 512], f32, tag="ppo", name="ppo") for _ in range(2)]

                def consume(kc, pm):
                    for qq in range(2):
                        nc.tensor.matmul(
                            o_ps[qq],
                            lhsT=v_sb[kc][:, h * 128:(h + 1) * 128],
                            rhs=pm[:, qq * 512:(qq + 1) * 512],
                            start=(kc == 0),
                            stop=(kc == NKC - 1),
                        )

                def d_mm(pr, pacc):
                    for qq in range(2):
                        nc.tensor.matmul(
                            d_ps[qq],
                            lhsT=ones128,
                            rhs=pacc[:, qq * 512:(qq + 1) * 512],
                            start=(pr == 0),
                            stop=(pr == NKC // 2 - 1),
                        )

                pending = []      # [(kc, pm)] — 2-stage consume delay
                pairs = [None] * (NKC // 2)   # accumulated pm pairs
                d_emit = []       # pairs ready to ones-matmul
                pm_even = None
                for kc in range(NKC):
                    sT = pp_big.tile([128, 1024], f32, tag="ppbig", name="ppbig")
                    for nn in range(2):
                        nc.tensor.matmul(
                            sT[:, nn * 512:(nn + 1) * 512],
                            lhsT=kT_sb[h][:, kc * 128:(kc + 1) * 128],
                            rhs=qT_sb[h][:, q0 + nn * 512:q0 + (nn + 1) * 512],
                            start=True,
                            stop=True,
                        )
                    p = p_pm.tile([128, 1024], bf16, tag="pm", name="pm")
                    nc.scalar.activation(out=p, in_=sT, func=Exp, scale=SCALE)
                    pm = p_pm.tile([128, 1024], bf16, tag="pm", name="pm")
                    nc.vector.tensor_mul(pm, p, keep_sl(kc, q0, q0 + 1024))

                    if kc % 2 == 0:
                        pm_even = pm
                    else:
                        pr = kc // 2
                        pacc = p_acc.tile([128, 1024], bf16, tag="acc", name="acc")
                        eng = nc.gpsimd if pr < gps_pairs else nc.vector
                        eng.tensor_add(pacc, pm_even, pm)
                        pairs[pr] = pacc
                        d_emit.append(pr)

                    pending.append((kc, pm))
                    if len(pending) > 3:
                        consume(*pending.pop(0))
                    # ones-matmul a pair two k-chunks after it was formed so
                    # the PE never waits on the DVE adds
                    if d_emit and d_emit[0] <= (kc - 3) // 2:
                        pr = d_emit.pop(0)
                        d_mm(pr, pairs[pr])
                    if kc == 2 and finish_prev is not None:
                        finish_prev()
                    # previous half's out-projection chunks, mid-loop: fills
                    # ACT-lag windows with PE work, and keeps their DVE cast
                    # out of the boundary sT rotation (was a 1.25us PE stall)
                    if interleave and kc in (6, 11):
                        out_proj(interleave.pop(0))
                for item in pending:
                    consume(*item)
                for pr in d_emit:
                    d_mm(pr, pairs[pr])
                while interleave:
                    out_proj(interleave.pop(0))

                def finish():
                    for qq in range(2):
                        # 1/d via exp(-ln(d)) on ACT: frees the PSUM
                        # accumulators fast and keeps DVE reciprocal (which
                        # measures ~6 cyc/elem) off the critical path.
                        lnd = p_sm.tile([128, 512], f32, tag="lnd", name="lnd")
                        nc.scalar.activation(out=lnd, in_=d_ps[qq], func=Ln)
                        rdb = p_sm.tile([128, 512], f32, tag="rdb", name="rdb")
                        nc.scalar.activation(out=rdb, in_=lnd, func=Exp, scale=-1.0)
                        nc.vector.tensor_mul(
                            oT_sb[h][:, q0 + qq * 512:q0 + (qq + 1) * 512],
                            o_ps[qq],
                            rdb,
                        )

                return finish

            fin = None
            for h in range(HPG):
                fin = attention_hh(h, 0, [], fin)
            sc_queue = list(range(8))
            for h in range(HPG):
                fin = attention_hh(h, 1, [sc_queue.pop(0), sc_queue.pop(0)], fin)
            fin()
            for sc in range(8, NSC):
                out_proj(sc, on_act=True, tail=True)

    _split_waits(nc, mybir, maxw=1)
    return nc


def _prep_core_inputs(x, attn_mask, qkv_w, qkv_b, q_w, q_b, k_w, k_b, v_w, v_b,
                      out_w, use_dr=USE_DR):
    """Host-side: fold projections, shard, pre-transpose/tile, cast."""
    f = np.float32
    x = np.asarray(x, f)
    qkv_w = np.asarray(qkv_w, f)
    qkv_b = np.asarray(qkv_b, f)
    Ws = {}
    bs = {}
    for i, (w, b) in enumerate(((q_w, q_b), (k_w, k_b), (v_w, v_b))):
        w = np.asarray(w, f)
        b = np.asarray(b, f)
        sl = slice(i * D, (i + 1) * D)
        Ws[i] = w @ qkv_w[sl]              # [D, D] effective
        bs[i] = b + w @ qkv_b[sl]          # [D]
    out_wT = np.ascontiguousarray(np.asarray(out_w, f).T)  # [D(hd), D(model)]

    keepT = (np.asarray(attn_mask).T == 0).astype(BF16)    # [k, q]
    keepT_t = np.ascontiguousarray(keepT).reshape(NKC, 128, S)

    def dr_pack(mat_T, scale):
        # mat_T: [D, N] (contraction-major) -> [NDP, 128, 2, N] e4m3 * scale
        m = (mat_T * scale).reshape(NDP, 2, 128, -1).transpose(0, 2, 1, 3)
        return np.ascontiguousarray(m.astype(F8E4))

    xT_all = []
    x8_all = []
    for b_i in range(B):
        xb = np.ascontiguousarray(x[b_i].T)                # [D, S] f32
        xT_all.append(np.ascontiguousarray(xb.astype(BF16)).reshape(ND, 128, S))
        if use_dr:
            x8_all.append(dr_pack(xb, X_SCALE))

    maps = []
    for c in range(8):
        b_i = c % B
        g = c // B
        sl = slice(g * GD, (g + 1) * GD)
        m = {
            "xT": xT_all[b_i],
            "wvT": np.ascontiguousarray(Ws[2][sl].T.astype(BF16)).reshape(ND, 128, GD),
            "bq": np.ascontiguousarray(bs[0][sl].reshape(HPG, 128).T.astype(f)),
            "bk": np.ascontiguousarray(bs[1][sl].reshape(HPG, 128).T.astype(f)),
            "bvrow": bs[2][sl].astype(BF16).reshape(1, GD),
            "outwT": np.ascontiguousarray(out_wT[sl].astype(BF16)).reshape(HPG, 128, D),
            "keepT": keepT_t,
        }
        if use_dr:
            m["x8"] = x8_all[b_i]
            m["wq8"] = dr_pack(np.ascontiguousarray(Ws[0][sl].T), W_SCALE)
            m["wk8"] = dr_pack(np.ascontiguousarray(Ws[1][sl].T), W_SCALE)
        else:
            m["wqT"] = np.ascontiguousarray(Ws[0][sl].T.astype(BF16)).reshape(ND, 128, GD)
            m["wkT"] = np.ascontiguousarray(Ws[1][sl].T.astype(BF16)).reshape(ND, 128, GD)
        maps.append(m)
    return maps


def kernel(x, attn_mask, qkv_w, qkv_b, q_w, q_b, k_w, k_b, v_w, v_b,
           out_w, out_b, _trace=False):
    _install_ntff_hook_shim()
    from concourse.bass_utils import run_bass_kernel_spmd

    in_maps = _prep_core_inputs(
        x, attn_mask, qkv_w, qkv_b, q_w, q_b, k_w, k_b, v_w, v_b, out_w
    )
    use_vbias = bool(np.any(np.asarray(in_maps[0]["bvrow"], np.float32) != 0))
    key = ("nc", use_vbias)
    if key not in _cached:
        _cached[key] = _build_program(use_vbias=use_vbias)
    nc = _cached[key]
    core_ids = list(range(8))
    try:
        res = run_bass_kernel_spmd(nc, in_maps, core_ids, trace=_trace)
    except Exception:
        # transient NRT device wedge recovers on retry
        res = run_bass_kernel_spmd(nc, in_maps, core_ids, trace=_trace)
    _cached["last_result"] = res

    out_b = np.asarray(out_b, np.float32)
    full = np.empty((B, S, D), np.float32)
    for b_i in range(B):
        full[b_i] = (
            res.results[b_i]["out"].astype(np.float32)
            + res.results[b_i + B]["out"].astype(np.float32)
            + out_b
        )
    return full
